# revision 1
# baseline (speedup 1.0000x reference)
"""BinaryDilGroupConv Trainium2 kernel.

Computes, for x[N=64, C=256, 32, 32]:
    h = BN(x)  (inference affine)
    a = sign(h); w = sign(weight)
    y = grouped dilated conv(a, w; groups=64, k=3, dil=2, pad=2)
    out = channel_shuffle(y, g=64) + x

Sharding: data-parallel over batch N across 8 NeuronCores (8 samples/core).
Params replicated. No collectives.

Device mapping (per core, per sample):
  - ACT: a = Sign(x*scale + bias) per 128-channel half, written fp8 into
    the interior of a zero-bordered padded tile (row pitch 40 bytes).
  - PE: grouped conv as block-diagonal matmuls: lhsT is [128 cin, 128
    cout] fp8 (zero off the 4x4 group diagonal), dilation handled by
    flat shifted-window reads of the padded tile. The dy=0/dy=1 tap
    pairs run as fp8 DoubleRow matmuls (pair stride 2 rows = 80B), the
    dy=2 taps as plain fp8 matmuls; 6 matmuls accumulate per PSUM tile
    of ny x 40 columns (x >= 32 columns are discarded as junk). PSUM
    partition order is m = 32j + g for conv cout 4g + j (lhsT columns
    permuted on the host) so the shuffle below uses contiguous blocks.
  - DVE: evict PSUM (keeping x < 32) to int8 (conv outputs are small
    integers, exact), DMA-permute the int8 conv tensor into shuffled
    (natural output) channel order, then fin = x + conv_perm.
  - Load and store are contiguous identity DMAs; the host pre/post
    reshapes (free) so DRAM runs are 8KB/partition.
"""

import numpy as np
import ml_dtypes

C = 256
G = 64            # groups
CPG = 4           # channels per group
K = 3
DIL = 2
PAD = 2
EPS = 1e-5
H = W = 32
S = H * W         # 1024 spatial positions
PH = 38           # padded rows (36 used + 2 spill rows for flat windows)
PW = 40           # padded cols (36 used + 4: row pitch 40B makes the
                  # DoubleRow pair stride 80B, a multiple of 16)
N_FULL = 64
N_CORES = 8
NS = N_FULL // N_CORES   # samples per core
NHALF = 2                # channel halves of 128
CHUNKS = [(0, 12), (12, 12), (24, 8)]  # (y0, ny): ny*40 <= 512 psum bank
ABUFS = 3                # padded-activation round-robin depth

_COMPILED = None


def build(n_samples=NS):
    """Build + compile the per-core Bass program."""
    import concourse.bass as bass
    import concourse.bacc as bacc
    import concourse.tile as tile
    import concourse.mybir as mybir

    fp32 = mybir.dt.float32
    fp8 = mybir.dt.float8e4
    i8 = mybir.dt.int8

    nc = bacc.Bacc("TRN2", target_bir_lowering=False, debug=False,
                   num_devices=N_CORES)

    # partition-major layouts so load/store DMAs are contiguous
    xin = nc.dram_tensor("xin", [n_samples, 128, NHALF, S], fp32,
                         kind="ExternalInput").ap()
    # weight free index = h*9 + dx*3 + slot (slot 0/1 = dy 0/1 pair
    # members, slot 2 = dy 2 single)
    wT = nc.dram_tensor("wT", [128, NHALF * K * K, 128], fp8,
                        kind="ExternalInput").ap()
    bnsc = nc.dram_tensor("bnsc", [NHALF, 128], fp32,
                          kind="ExternalInput").ap()
    bnbi = nc.dram_tensor("bnbi", [NHALF, 128], fp32,
                          kind="ExternalInput").ap()
    out = nc.dram_tensor("out", [n_samples, 128, NHALF, S], fp32,
                         kind="ExternalOutput").ap()

    with tile.TileContext(nc) as tc:
        with (
            tc.tile_pool(name="const", bufs=1) as constp,
            tc.tile_pool(name="xp", bufs=n_samples) as xp,
            tc.tile_pool(name="ci8p", bufs=4) as ci8p,
            tc.tile_pool(name="cpermp", bufs=3) as cpermp,
            tc.tile_pool(name="finp", bufs=3) as finp,
            tc.tile_pool(name="psum", bufs=8, space="PSUM") as psump,
        ):
            # ---- first x load + BN params go first so the first Sign
            # starts ASAP; weights overlap with it
            x_nats = {}

            def load_x(n):
                x_nats[n] = xp.tile([128, NHALF, S], fp32, name="x_nat",
                                    tag="x_nat")
                nc.sync.dma_start(x_nats[n][:], xin[n])

            # x0 on the SP DGE ring; params on the ACT DGE ring so they
            # transfer in parallel with x0
            load_x(0)
            sc_tile = constp.tile([128, NHALF], fp32)
            nc.scalar.dma_start(sc_tile[:], bnsc.rearrange("h p -> p h"))
            bi_tile = constp.tile([128, NHALF], fp32)
            nc.scalar.dma_start(bi_tile[:], bnbi.rearrange("h p -> p h"))
            w_tile = constp.tile([128, NHALF * K * K, 128], fp8)
            nc.scalar.dma_start(w_tile[:], wT)

            # warmup: trigger the ACT table load early and keep the PE
            # HAM window busy until the real stream starts. The second
            # batch reads the real weight tile so it runs right before
            # the first real matmul (bridging the HAM activity window).
            warm_sb = constp.tile([128, 512], fp8)
            nc.gpsimd.memset(warm_sb[:], 0.0)
            warm_w = constp.tile([128, 128], fp8)
            nc.gpsimd.memset(warm_w[:], 0.0)
            warm_act = constp.tile([128, 16], fp8)
            nc.scalar.activation(warm_act[:], warm_sb[:, 0:16],
                                 mybir.ActivationFunctionType.Sign)
            for _ in range(12):
                wps = psump.tile([128, 512], fp32, name="ps", tag="ps")
                nc.tensor.matmul(wps[:], warm_w[:], warm_sb[:],
                                 start=True, stop=True)
            for _ in range(8):
                wps = psump.tile([128, 512], fp32, name="ps", tag="ps")
                nc.tensor.matmul(wps[:], w_tile[:, 0, :], warm_sb[:],
                                 start=True, stop=True)

            # ---- persistent padded activation tiles, borders zeroed once
            a_pads = [[constp.tile([128, PH * PW], fp8,
                                   name=f"apad{h}_{b}")
                       for b in range(ABUFS)] for h in range(NHALF)]
            for h in range(NHALF):
                for b in range(ABUFS):
                    ap3 = a_pads[h][b][:].rearrange("p (y x) -> p y x", x=PW)
                    nc.gpsimd.memset(ap3[:, 0:PAD, :], 0.0)
                    nc.gpsimd.memset(ap3[:, PAD + H:PH, :], 0.0)
                    nc.gpsimd.memset(ap3[:, PAD:PAD + H, 0:PAD], 0.0)
                    nc.gpsimd.memset(ap3[:, PAD:PAD + H, PAD + W:PW], 0.0)

            # ---- front-load ALL remaining x loads: the DMA engines run
            # at the edge of saturation during the matmul stream, so the
            # input traffic is moved to the (DMA-idle) prologue
            for n in range(1, n_samples):
                load_x(n)

            # deferred-by-one-sample ACT-side permutes + residual add +
            # store, so no engine FIFO makes sample n+1's work wait on
            # sample n's permute chain
            deferred = {}
            conv_perm_of = {}

            def perm_dma(eng, n, h, j):
                _, _, conv_i8_n = deferred[n]
                eng.dma_start(
                    conv_perm_of[n][64 * (j % 2) + 32 * h:
                                    64 * (j % 2) + 32 * h + 32,
                                    j // 2, :],
                    conv_i8_n[h][32 * j:32 * j + 32, :],
                )

            def finish_sample(n, fine=False):
                x_nat_n, conv_perm_n, _ = deferred[n]
                for j in (2, 3):
                    for h in range(NHALF):
                        perm_dma(nc.sync, n, h, j)
                fin = finp.tile([128, NHALF, S], fp32, name="fin",
                                tag="fin")
                # fine=True splits the adds/stores so the final sample's
                # tail chain pipelines instead of serializing
                splits = 2 if fine else 1
                for hh in range(NHALF):
                    for q in range(splits):
                        sl = slice(q * S // splits, (q + 1) * S // splits)
                        nc.vector.tensor_add(
                            fin[:, hh, sl], x_nat_n[:, hh, sl],
                            conv_perm_n[:, hh, sl])
                        nc.sync.dma_start(out[n][:, hh, sl],
                                          fin[:, hh, sl])
                deferred.pop(n)
                conv_perm_of.pop(n)

            def window(apad, offset, rsteps, ncols):
                """Flat shifted-window AP [128, rsteps?, ncols] of the
                padded activation tile (manual AP: the pair dim strides
                2 rows = 80 elements, not expressible by rearrange)."""
                base = apad[:, offset:offset + 1]
                ap = [list(apad[:].ap[0])]
                if rsteps:
                    ap.append([2 * PW, rsteps])
                ap.append([1, ncols])
                return bass.AP(base.tensor, base.offset, ap)

            for n in range(n_samples):
                x_nat = x_nats.pop(n)

                # ---- a = Sign(x*scale + bias), fp8, into padded interior
                for h in range(NHALF):
                    ap3 = a_pads[h][n % ABUFS][:].rearrange(
                        "p (y x) -> p y x", x=PW)
                    nc.scalar.activation(
                        ap3[:, PAD:PAD + H, PAD:PAD + W],
                        x_nat[:, h, :].rearrange("p (y x) -> p y x", x=W),
                        mybir.ActivationFunctionType.Sign,
                        bias=bi_tile[:, h:h + 1],
                        scale=sc_tile[:, h:h + 1],
                    )

                # ---- conv: fp8 DoubleRow pairs + singles per chunk,
                # evicted (x < 32 only) to int8
                conv_i8 = [ci8p.tile([128, S], i8, name=f"ci8_{h}",
                                     tag=f"ci8_{h}") for h in range(NHALF)]
                for h in range(NHALF):
                    apad = a_pads[h][n % ABUFS]
                    for (y0, ny) in CHUNKS:
                        N = ny * PW
                        ps = psump.tile([128, N], fp32, name="ps", tag="ps")
                        for dx in range(K):
                            wi = h * K * K + dx * K
                            nc.tensor.matmul(
                                ps[:],
                                w_tile[:, wi:wi + 2, :],
                                window(apad, y0 * PW + DIL * dx, 2, N),
                                start=(dx == 0), stop=False,
                                perf_mode=mybir.MatmulPerfMode.DoubleRow,
                            )
                        for dx in range(K):
                            wi = h * K * K + dx * K + 2
                            nc.tensor.matmul(
                                ps[:],
                                w_tile[:, wi, :],
                                window(apad, (y0 + 2 * DIL) * PW + DIL * dx,
                                       0, N),
                                start=False, stop=(dx == K - 1),
                            )
                        nc.vector.tensor_copy(
                            conv_i8[h][:, y0 * W:(y0 + ny) * W].rearrange(
                                "p (y x) -> p y x", x=W),
                            ps[:].rearrange("p (y x) -> p y x",
                                            x=PW)[:, :, 0:W],
                        )

                # ---- shuffle-permute the int8 conv into natural final
                # channel order: psum (m=32j+g, half h) holds conv cout
                # 4g+j -> final channel 64j+32h+g = (slot j//2,
                # partition 64*(j%2)+32h+g).
                conv_perm = cpermp.tile([128, NHALF, S], i8)
                deferred[n] = (x_nat, conv_perm, conv_i8)
                conv_perm_of[n] = conv_perm
                for j in (0, 1):
                    for h in range(NHALF):
                        perm_dma(nc.gpsimd, n, h, j)

                # ---- rest of permute + add + store for PREVIOUS sample
                if n > 0:
                    finish_sample(n - 1)
            finish_sample(n_samples - 1, fine=True)

    nc.compile()
    return nc


def _host_prep(x, weight, gamma, beta, running_mean, running_var):
    """Precompute BN affine + block-diagonal signed weights."""
    inv = (gamma / np.sqrt(running_var + EPS)).astype(np.float32)
    bias = (beta - running_mean * inv).astype(np.float32)
    wsign = np.sign(weight).astype(np.float32)   # [256, 4, 3, 3]

    lhsT = np.zeros((NHALF, K * K, 128, 128), np.float32)
    # Column m of lhsT (-> PSUM partition m) holds cout co = 4*(m%32)+m//32
    # within the half, so PSUM partition order is m = 32j + g for conv
    # cout 4g + j (see the device-side comment on conv_perm).
    m = np.arange(128)
    co = CPG * (m % 32) + m // 32
    gl = co // CPG
    for h in range(NHALF):
        for dy in range(K):
            for dx in range(K):
                # device tap index: dx*3 + dy (dy 0/1 = DoubleRow pair)
                t = dx * K + dy
                for kk in range(CPG):
                    lhsT[h, t, CPG * gl + kk, m] = wsign[128 * h + co, kk,
                                                         dy, dx]
    # device weight layout: [ci, (h,t), m], fp8, contiguous upload
    lhsT = np.ascontiguousarray(
        lhsT.astype(ml_dtypes.float8_e4m3)
        .transpose(2, 0, 1, 3)
        .reshape(128, NHALF * K * K, 128))
    sc = np.ascontiguousarray(inv.reshape(NHALF, 128))
    bi = np.ascontiguousarray(bias.reshape(NHALF, 128))
    return lhsT, sc, bi


def _get_compiled():
    global _COMPILED
    if _COMPILED is None:
        _COMPILED = build(NS)
    return _COMPILED


def make_in_maps(x, weight, gamma, beta, running_mean, running_var):
    lhsT, sc, bi = _host_prep(x, weight, gamma, beta, running_mean,
                              running_var)
    # [cores, ns, 2, 128, S] -> partition-major [cores, ns, 128, 2, S]
    xs = np.ascontiguousarray(
        x.astype(np.float32)
        .reshape(N_CORES, NS, NHALF, 128, S)
        .transpose(0, 1, 3, 2, 4))
    return [
        {"xin": xs[i], "wT": lhsT, "bnsc": sc, "bnbi": bi}
        for i in range(N_CORES)
    ]


def kernel(x, weight, gamma, beta, running_mean, running_var):
    from concourse.bass_utils import run_bass_kernel_spmd

    nc = _get_compiled()
    in_maps = make_in_maps(np.asarray(x), np.asarray(weight),
                           np.asarray(gamma), np.asarray(beta),
                           np.asarray(running_mean), np.asarray(running_var))
    res = run_bass_kernel_spmd(nc, in_maps, list(range(N_CORES)))
    # device out is [ns, 128, 2, S] partition-major; channel c' = 128*slot+p
    outs = [res.results[i]["out"].transpose(0, 2, 1, 3).reshape(NS, C, H, W)
            for i in range(N_CORES)]
    return np.concatenate(outs, axis=0).astype(np.float32)



# revision 5
# speedup vs baseline: 1.3100x; 1.3100x over previous
"""BinaryDilGroupConv Trainium2 kernel (v2).

Computes, for x[N=64, C=256, 32, 32]:
    h = BN(x)  (inference affine)
    a = sign(h); w = sign(weight)
    y = grouped dilated conv(a, w; groups=64, k=3, dil=2, pad=2)
    out = channel_shuffle(y, g=64) + x

Sharding: data-parallel over batch N across 8 NeuronCores (8 samples/core).
Params replicated. No collectives.

v2 design (vs v1 baseline):
  - The channel shuffle + residual + PSUM eviction are FUSED into one DVE
    tensor_add per (sample, half): psum already holds couts in an order
    (m = 32j + g for conv cout 4g+j) where the shuffled final channel is
    64j + 32h + g, so a SECOND copy of x loaded with exactly that channel
    permutation (one strided DMA from the same DRAM tensor) lines up
    partition-for-partition with psum. fin = psum + x_res, written bf16.
    The store DMA then scatters 32-partition blocks back to natural
    channel order. This deletes v1's separate CAST eviction, int8
    permute DMAs, and fp32 adds - and the 30us+ post-matmul tail.
  - x for the sign path stays fp32 (bit-exact signs vs the reference);
    the residual copy and the output are bf16 (rel err ~2e-3 << 2e-2).
  - Matmuls are tap-major per half-sample so consecutive MMs share lhsT
    (fewer LDWEIGHTS stalls); uniform 4 chunks x 8 rows, one 4-bank psum
    tile per half. The 3 dy=2 single-tap matmuls use exact 2D windows
    (256 cols, no junk); only the DoubleRow pairs need the flat 40-wide
    window (pair step 80B must be 16B-aligned).
"""

import numpy as np
import ml_dtypes

C = 256
G = 64            # groups
CPG = 4           # channels per group
K = 3
DIL = 2
PAD = 2
EPS = 1e-5
H = W = 32
S = H * W         # 1024 spatial positions
PH = 38           # padded rows (36 used + 2 spill rows for flat windows)
PW = 40           # padded cols (36 used + 4: row pitch 40B makes the
                  # DoubleRow pair stride 80B, a multiple of 16)
N_FULL = 64
N_CORES = 8
NS = N_FULL // N_CORES   # samples per core
NHALF = 2                # channel halves of 128
NCHUNK = 4               # psum chunks per half (8 output rows each)
CROWS = H // NCHUNK      # 8 rows per chunk
NFLAT = CROWS * PW       # 320 flat columns per DR matmul
ABUFS = 3                # padded-activation round-robin depth

_COMPILED = None


def build(n_samples=NS):
    """Build + compile the per-core Bass program."""
    import concourse.bass as bass
    import concourse.bacc as bacc
    import concourse.tile as tile
    import concourse.mybir as mybir

    fp32 = mybir.dt.float32
    bf16 = mybir.dt.bfloat16
    fp8 = mybir.dt.float8e4

    nc = bacc.Bacc("TRN2", target_bir_lowering=False, debug=False,
                   num_devices=N_CORES)

    # natural channel order; loaded twice with different channel->partition
    # mappings (natural for the sign path, shuffled for the residual)
    xin = nc.dram_tensor("xin", [n_samples, C, S], fp32,
                         kind="ExternalInput").ap()
    xres = nc.dram_tensor("xres", [n_samples, C, S], bf16,
                          kind="ExternalInput").ap()
    # weight free index = h*9 + dx*3 + slot (slot 0/1 = dy 0/1 pair
    # members, slot 2 = dy 2 single)
    wT = nc.dram_tensor("wT", [128, NHALF * K * K, 128], fp8,
                        kind="ExternalInput").ap()
    bnsc = nc.dram_tensor("bnsc", [NHALF, 128], fp32,
                          kind="ExternalInput").ap()
    bnbi = nc.dram_tensor("bnbi", [NHALF, 128], fp32,
                          kind="ExternalInput").ap()
    out = nc.dram_tensor("out", [n_samples, C, S], bf16,
                         kind="ExternalOutput").ap()

    with tile.TileContext(nc) as tc:
        with (
            tc.tile_pool(name="const", bufs=1) as constp,
            tc.tile_pool(name="xnp", bufs=n_samples) as xnp,
            tc.tile_pool(name="xrp", bufs=n_samples) as xrp,
            tc.tile_pool(name="finp", bufs=3) as finp,
            tc.tile_pool(name="psum", bufs=2, space="PSUM") as psump,
        ):
            # ---- first x load + BN params go first so the first Sign
            # starts ASAP; weights overlap with it
            xn_t = {}
            xr_t = {}

            def load_xn(n):
                xn_t[n] = xnp.tile([128, NHALF, S], fp32, name="xn",
                                   tag="xn")
                nc.sync.dma_start(xn_t[n][:],
                                  xin[n].rearrange("(h p) s -> p h s",
                                                   p=128))

            def load_xr(n):
                xr_t[n] = xrp.tile([128, NHALF, S], bf16, name="xr",
                                   tag="xr")
                # partition 32j+g, slot h  <-  channel 64j + 32h + g
                for j in range(4):
                    nc.gpsimd.dma_start(
                        xr_t[n][32 * j:32 * j + 32, :, :],
                        xres[n, 64 * j:64 * j + 64, :].rearrange(
                            "(h g) s -> g h s", h=2))

            load_xn(0)
            sc_tile = constp.tile([128, NHALF], fp32)
            nc.scalar.dma_start(sc_tile[:], bnsc.rearrange("h p -> p h"))
            bi_tile = constp.tile([128, NHALF], fp32)
            nc.scalar.dma_start(bi_tile[:], bnbi.rearrange("h p -> p h"))
            w_tile = constp.tile([128, NHALF * K * K, 128], fp8)
            nc.scalar.dma_start(w_tile[:], wT)

            # warmup: trigger the ACT table load early and keep the PE
            # busy until the first real matmul (so HAM is at K=8/8 by
            # then). Second batch reads the real weight tile.
            warm_sb = constp.tile([128, 512], fp8)
            nc.gpsimd.memset(warm_sb[:], 0.0)
            warm_w = constp.tile([128, 128], fp8)
            nc.gpsimd.memset(warm_w[:], 0.0)
            warm_act = constp.tile([128, 16], fp8)
            nc.scalar.activation(warm_act[:], warm_sb[:, 0:16],
                                 mybir.ActivationFunctionType.Sign)
            warm_ps = psump.tile([128, NCHUNK, 512], fp32, name="ps",
                                 tag="ps")
            for i in range(5):
                nc.tensor.matmul(warm_ps[:, i % NCHUNK, :], warm_w[:],
                                 warm_sb[:], start=True, stop=True)
            for i in range(4):
                nc.tensor.matmul(warm_ps[:, i % NCHUNK, :],
                                 w_tile[:, 0, :], warm_sb[:],
                                 start=True, stop=True)

            # ---- persistent padded activation tiles, borders zeroed once
            a_pads = [[constp.tile([128, PH * PW], fp8,
                                   name=f"apad{h}_{b}")
                       for b in range(ABUFS)] for h in range(NHALF)]
            for h in range(NHALF):
                for b in range(ABUFS):
                    ap3 = a_pads[h][b][:].rearrange("p (y x) -> p y x", x=PW)
                    nc.gpsimd.memset(ap3[:, 0:PAD, :], 0.0)
                    nc.gpsimd.memset(ap3[:, PAD + H:PH, :], 0.0)
                    nc.gpsimd.memset(ap3[:, PAD:PAD + H, 0:PAD], 0.0)
                    nc.gpsimd.memset(ap3[:, PAD:PAD + H, PAD + W:PW], 0.0)

            # ---- front-load the remaining input DMAs (input rings are
            # otherwise idle during the matmul stream)
            load_xr(0)
            for n in range(1, n_samples):
                load_xn(n)
                load_xr(n)

            def window(apad, offset, rsteps, ncols):
                """Flat shifted-window AP [128, rsteps?, ncols] of the
                padded activation tile (manual AP: the pair dim strides
                2 rows = 80 elements, not expressible by rearrange)."""
                base = apad[:, offset:offset + 1]
                ap = [list(apad[:].ap[0])]
                if rsteps:
                    ap.append([2 * PW, rsteps])
                ap.append([1, ncols])
                return bass.AP(base.tensor, base.offset, ap)

            def window2d(apad, offset):
                """Exact [128, CROWS, W] window (row-jumping, no junk)."""
                base = apad[:, offset:offset + 1]
                ap = [list(apad[:].ap[0]), [PW, CROWS], [1, W]]
                return bass.AP(base.tensor, base.offset, ap)

            for n in range(n_samples):
                xn = xn_t.pop(n)
                xr = xr_t.pop(n)

                # ---- a = Sign(x*scale + bias), fp8, into padded interior
                for h in range(NHALF):
                    ap3 = a_pads[h][n % ABUFS][:].rearrange(
                        "p (y x) -> p y x", x=PW)
                    nc.scalar.activation(
                        ap3[:, PAD:PAD + H, PAD:PAD + W],
                        xn[:, h, :].rearrange("p (y x) -> p y x", x=W),
                        mybir.ActivationFunctionType.Sign,
                        bias=bi_tile[:, h:h + 1],
                        scale=sc_tile[:, h:h + 1],
                    )

                # ---- conv: tap-major per half so consecutive MMs share
                # lhsT. 6 passes: 3 fp8 DoubleRow (dy0+dy1 per dx, flat
                # 320-col windows) + 3 singles (dy2 per dx, exact 2D
                # 256-col windows). 4 chunks of 8 output rows, each into
                # its own psum bank slice.
                ps_h = []
                for h in range(NHALF):
                    apad = a_pads[h][n % ABUFS]
                    ps = psump.tile([128, NCHUNK, 512], fp32, name="ps",
                                    tag="ps")
                    ps_h.append(ps)
                    for dx in range(K):
                        wi = h * K * K + dx * K
                        for c in range(NCHUNK):
                            nc.tensor.matmul(
                                ps[:, c, 0:NFLAT],
                                w_tile[:, wi:wi + 2, :],
                                window(apad, (CROWS * c) * PW + DIL * dx,
                                       2, NFLAT),
                                start=(dx == 0), stop=False,
                                perf_mode=mybir.MatmulPerfMode.DoubleRow,
                            )
                    for dx in range(K):
                        wi = h * K * K + dx * K + 2
                        for c in range(NCHUNK):
                            dst = ps[:, c, 0:NFLAT].rearrange(
                                "p (y x) -> p y x", x=PW)[:, :, 0:W]
                            nc.tensor.matmul(
                                dst,
                                w_tile[:, wi, :],
                                window2d(apad,
                                         (CROWS * c + 2 * DIL) * PW
                                         + DIL * dx),
                                start=False, stop=(dx == K - 1),
                            )

                # ---- fused eviction + shuffle + residual: fin (bf16) =
                # psum (keep x<32 of each 40-col row) + x_res
                fin = finp.tile([128, NHALF, S], bf16, name="fin",
                                tag="fin")
                for h in range(NHALF):
                    keep = ps_h[h][:, :, 0:NFLAT].rearrange(
                        "p c (y x) -> p c y x", x=PW)[:, :, :, 0:W]
                    nc.vector.tensor_add(
                        fin[:, h, :].rearrange("p (c y x) -> p c y x",
                                               c=NCHUNK, x=W),
                        keep,
                        xr[:, h, :].rearrange("p (c y x) -> p c y x",
                                              c=NCHUNK, x=W),
                    )

                # ---- store: scatter 32-partition blocks back to natural
                # channel order (channel 64j + 32h + g <- partition 32j+g,
                # slot h)
                for j in range(4):
                    nc.sync.dma_start(
                        out[n, 64 * j:64 * j + 64, :].rearrange(
                            "(h g) s -> g h s", h=2),
                        fin[32 * j:32 * j + 32, :, :])

    nc.compile()
    return nc


def _host_prep(x, weight, gamma, beta, running_mean, running_var):
    """Precompute BN affine + block-diagonal signed weights."""
    inv = (gamma / np.sqrt(running_var + EPS)).astype(np.float32)
    bias = (beta - running_mean * inv).astype(np.float32)
    wsign = np.sign(weight).astype(np.float32)   # [256, 4, 3, 3]

    lhsT = np.zeros((NHALF, K * K, 128, 128), np.float32)
    # Column m of lhsT (-> PSUM partition m) holds cout co = 4*(m%32)+m//32
    # within the half, so PSUM partition order is m = 32j + g for conv
    # cout 4g + j; the shuffled final channel is then 64j + 32h + g.
    m = np.arange(128)
    co = CPG * (m % 32) + m // 32
    gl = co // CPG
    for h in range(NHALF):
        for dy in range(K):
            for dx in range(K):
                # device tap index: dx*3 + dy (dy 0/1 = DoubleRow pair)
                t = dx * K + dy
                for kk in range(CPG):
                    lhsT[h, t, CPG * gl + kk, m] = wsign[128 * h + co, kk,
                                                         dy, dx]
    # device weight layout: [ci, (h,t), m], fp8, contiguous upload
    lhsT = np.ascontiguousarray(
        lhsT.astype(ml_dtypes.float8_e4m3)
        .transpose(2, 0, 1, 3)
        .reshape(128, NHALF * K * K, 128))
    sc = np.ascontiguousarray(inv.reshape(NHALF, 128))
    bi = np.ascontiguousarray(bias.reshape(NHALF, 128))
    return lhsT, sc, bi


def _get_compiled():
    global _COMPILED
    if _COMPILED is None:
        _COMPILED = build(NS)
    return _COMPILED


def make_in_maps(x, weight, gamma, beta, running_mean, running_var):
    lhsT, sc, bi = _host_prep(x, weight, gamma, beta, running_mean,
                              running_var)
    xs = np.ascontiguousarray(
        x.astype(np.float32).reshape(N_CORES, NS, C, S))
    xrs = np.ascontiguousarray(xs.astype(ml_dtypes.bfloat16))
    return [
        {"xin": xs[i], "xres": xrs[i], "wT": lhsT, "bnsc": sc, "bnbi": bi}
        for i in range(N_CORES)
    ]


def kernel(x, weight, gamma, beta, running_mean, running_var):
    from concourse.bass_utils import run_bass_kernel_spmd

    nc = _get_compiled()
    in_maps = make_in_maps(np.asarray(x), np.asarray(weight),
                           np.asarray(gamma), np.asarray(beta),
                           np.asarray(running_mean), np.asarray(running_var))
    res = run_bass_kernel_spmd(nc, in_maps, list(range(N_CORES)))
    outs = [res.results[i]["out"].astype(np.float32)
            .reshape(NS, C, H, W) for i in range(N_CORES)]
    return np.concatenate(outs, axis=0)


# revision 12
# speedup vs baseline: 1.3173x; 1.0056x over previous
"""BinaryDilGroupConv Trainium2 kernel (v2).

Computes, for x[N=64, C=256, 32, 32]:
    h = BN(x)  (inference affine)
    a = sign(h); w = sign(weight)
    y = grouped dilated conv(a, w; groups=64, k=3, dil=2, pad=2)
    out = channel_shuffle(y, g=64) + x

Sharding: data-parallel over batch N across 8 NeuronCores (8 samples/core).
Params replicated. No collectives.

v2 design (vs v1 baseline):
  - The channel shuffle + residual + PSUM eviction are FUSED into one DVE
    tensor_add per (sample, half): psum already holds couts in an order
    (m = 32j + g for conv cout 4g+j) where the shuffled final channel is
    64j + 32h + g, so a SECOND copy of x loaded with exactly that channel
    permutation (one strided DMA from the same DRAM tensor) lines up
    partition-for-partition with psum. fin = psum + x_res, written bf16.
    The store DMA then scatters 32-partition blocks back to natural
    channel order. This deletes v1's separate CAST eviction, int8
    permute DMAs, and fp32 adds - and the 30us+ post-matmul tail.
  - x for the sign path stays fp32 (bit-exact signs vs the reference);
    the residual copy and the output are bf16 (rel err ~2e-3 << 2e-2).
  - Matmuls are tap-major per half-sample so consecutive MMs share lhsT
    (fewer LDWEIGHTS stalls); uniform 4 chunks x 8 rows, one 4-bank psum
    tile per half. The 3 dy=2 single-tap matmuls use exact 2D windows
    (256 cols, no junk); only the DoubleRow pairs need the flat 40-wide
    window (pair step 80B must be 16B-aligned).
"""

import numpy as np
import ml_dtypes

C = 256
G = 64            # groups
CPG = 4           # channels per group
K = 3
DIL = 2
PAD = 2
EPS = 1e-5
H = W = 32
S = H * W         # 1024 spatial positions
PH = 38           # padded rows (36 used + 2 spill rows for flat windows)
PW = 40           # padded cols (36 used + 4: row pitch 40B makes the
                  # DoubleRow pair stride 80B, a multiple of 16)
N_FULL = 64
N_CORES = 8
NS = N_FULL // N_CORES   # samples per core
NHALF = 2                # channel halves of 128
NCHUNK = 4               # psum chunks per half (8 output rows each)
CROWS = H // NCHUNK      # 8 rows per chunk
NFLAT = CROWS * PW       # 320 flat columns per DR matmul
ABUFS = 3                # padded-activation round-robin depth

_COMPILED = None


def build(n_samples=NS):
    """Build + compile the per-core Bass program."""
    import concourse.bass as bass
    import concourse.bacc as bacc
    import concourse.tile as tile
    import concourse.mybir as mybir

    fp32 = mybir.dt.float32
    bf16 = mybir.dt.bfloat16
    fp16 = mybir.dt.float16
    fp8 = mybir.dt.float8e4

    nc = bacc.Bacc("TRN2", target_bir_lowering=False, debug=False,
                   num_devices=N_CORES)

    # natural channel order; loaded twice with different channel->partition
    # mappings (natural for the sign path, shuffled for the residual).
    # fp16 keeps BN sign flips negligible (~8 per sample vs ~70 for bf16).
    xin = nc.dram_tensor("xin", [n_samples, C, S], fp16,
                         kind="ExternalInput").ap()
    xres = nc.dram_tensor("xres", [n_samples, C, S], bf16,
                          kind="ExternalInput").ap()
    # weight free index = h*9 + dx*3 + slot (slot 0/1 = dy 0/1 pair
    # members, slot 2 = dy 2 single)
    wT = nc.dram_tensor("wT", [128, NHALF * K * K, 128], fp8,
                        kind="ExternalInput").ap()
    bnsc = nc.dram_tensor("bnsc", [NHALF, 128], fp32,
                          kind="ExternalInput").ap()
    bnbi = nc.dram_tensor("bnbi", [NHALF, 128], fp32,
                          kind="ExternalInput").ap()
    out = nc.dram_tensor("out", [n_samples, C, S], bf16,
                         kind="ExternalOutput").ap()

    with tile.TileContext(nc) as tc:
        with (
            tc.tile_pool(name="const", bufs=1) as constp,
            tc.tile_pool(name="xnp", bufs=n_samples) as xnp,
            tc.tile_pool(name="xrp", bufs=n_samples) as xrp,
            tc.tile_pool(name="finp", bufs=4) as finp,
            tc.tile_pool(name="psum", bufs=2, space="PSUM") as psump,
        ):
            # ---- first x load + BN params go first so the first Sign
            # starts ASAP; weights overlap with it
            xn_t = {}
            xr_t = {}

            def load_xn(n, split=False):
                xn_t[n] = xnp.tile([128, NHALF, S], fp16, name="xn",
                                   tag="xn")
                src = xin[n].rearrange("(h p) s -> p h s", p=128)
                if split:
                    # first sample: both halves in parallel on two rings
                    nc.sync.dma_start(xn_t[n][:, 0, :], src[:, 0, :])
                    nc.gpsimd.dma_start(xn_t[n][:, 1, :], src[:, 1, :])
                else:
                    nc.scalar.dma_start(xn_t[n][:], src)

            def load_xr(n):
                xr_t[n] = xrp.tile([128, NHALF, S], bf16, name="xr",
                                   tag="xr")
                # partition 32j+g, slot h  <-  channel 64j + 32h + g
                for j in range(4):
                    nc.gpsimd.dma_start(
                        xr_t[n][32 * j:32 * j + 32, :, :],
                        xres[n, 64 * j:64 * j + 64, :].rearrange(
                            "(h g) s -> g h s", h=2))

            load_xn(0, split=True)
            w_tile = constp.tile([128, NHALF * K * K, 128], fp8)
            nc.scalar.dma_start(w_tile[:], wT)
            sc_tile = constp.tile([128, NHALF], fp32)
            nc.scalar.dma_start(sc_tile[:], bnsc.rearrange("h p -> p h"))
            bi_tile = constp.tile([128, NHALF], fp32)
            nc.scalar.dma_start(bi_tile[:], bnbi.rearrange("h p -> p h"))

            # warmup: trigger the ACT table load early and keep the PE
            # busy until the first real matmul (so HAM is at K=8/8 by
            # then). Second batch reads the real weight tile.
            warm_sb = constp.tile([128, 512], fp8)
            nc.gpsimd.memset(warm_sb[:], 0.0)
            warm_w = constp.tile([128, 128], fp8)
            nc.gpsimd.memset(warm_w[:], 0.0)
            warm_act = constp.tile([128, 16], fp8)
            nc.scalar.activation(warm_act[:], warm_sb[:, 0:16],
                                 mybir.ActivationFunctionType.Sign)
            warm_ps = psump.tile([128, NCHUNK, 512], fp32, name="ps",
                                 tag="ps")
            for i in range(6):
                nc.tensor.matmul(warm_ps[:, i % NCHUNK, :], warm_w[:],
                                 warm_sb[:], start=True, stop=True)
            for i in range(4):
                nc.tensor.matmul(warm_ps[:, i % NCHUNK, :],
                                 w_tile[:, 0, :], warm_sb[:],
                                 start=True, stop=True)

            # ---- persistent padded activation tiles, borders zeroed once
            a_pads = [[constp.tile([128, PH * PW], fp8,
                                   name=f"apad{h}_{b}")
                       for b in range(ABUFS)] for h in range(NHALF)]
            for h in range(NHALF):
                for b in range(ABUFS):
                    ap3 = a_pads[h][b][:].rearrange("p (y x) -> p y x", x=PW)
                    nc.gpsimd.memset(ap3[:, 0:PAD, :], 0.0)
                    nc.gpsimd.memset(ap3[:, PAD + H:PH, :], 0.0)
                    nc.gpsimd.memset(ap3[:, PAD:PAD + H, 0:PAD], 0.0)
                    nc.gpsimd.memset(ap3[:, PAD:PAD + H, PAD + W:PW], 0.0)

            # ---- stagger the remaining input DMAs: keep ~3 samples of
            # lead so no ring backs up behind a full batch of loads
            load_xr(0)
            for n in (1, 2):
                load_xn(n)
                load_xr(n)

            def window(apad, offset, rsteps, ncols):
                """Flat shifted-window AP [128, rsteps?, ncols] of the
                padded activation tile (manual AP: the pair dim strides
                2 rows = 80 elements, not expressible by rearrange)."""
                base = apad[:, offset:offset + 1]
                ap = [list(apad[:].ap[0])]
                if rsteps:
                    ap.append([2 * PW, rsteps])
                ap.append([1, ncols])
                return bass.AP(base.tensor, base.offset, ap)

            def window2d(apad, offset):
                """Exact [128, CROWS, W] window (row-jumping, no junk)."""
                base = apad[:, offset:offset + 1]
                ap = [list(apad[:].ap[0]), [PW, CROWS], [1, W]]
                return bass.AP(base.tensor, base.offset, ap)

            for n in range(n_samples):
                if n + 3 < n_samples:
                    load_xn(n + 3)
                    load_xr(n + 3)
                xn = xn_t.pop(n)
                xr = xr_t.pop(n)

                # ---- a = Sign(x*scale + bias), fp8, into padded interior
                for h in range(NHALF):
                    ap3 = a_pads[h][n % ABUFS][:].rearrange(
                        "p (y x) -> p y x", x=PW)
                    nc.scalar.activation(
                        ap3[:, PAD:PAD + H, PAD:PAD + W],
                        xn[:, h, :].rearrange("p (y x) -> p y x", x=W),
                        mybir.ActivationFunctionType.Sign,
                        bias=bi_tile[:, h:h + 1],
                        scale=sc_tile[:, h:h + 1],
                    )

                # ---- conv: tap-major per half so consecutive MMs share
                # lhsT. 6 passes: 3 fp8 DoubleRow (dy0+dy1 per dx, flat
                # 320-col windows) + 3 singles (dy2 per dx, exact 2D
                # 256-col windows). 4 chunks of 8 output rows, each into
                # its own psum bank slice.
                ps_h = []
                for h in range(NHALF):
                    apad = a_pads[h][n % ABUFS]
                    ps = psump.tile([128, NCHUNK, 512], fp32, name="ps",
                                    tag="ps")
                    ps_h.append(ps)
                    for dx in range(K):
                        wi = h * K * K + dx * K
                        for c in range(NCHUNK):
                            nc.tensor.matmul(
                                ps[:, c, 0:NFLAT],
                                w_tile[:, wi:wi + 2, :],
                                window(apad, (CROWS * c) * PW + DIL * dx,
                                       2, NFLAT),
                                start=(dx == 0), stop=False,
                                perf_mode=mybir.MatmulPerfMode.DoubleRow,
                            )
                    for dx in range(K):
                        wi = h * K * K + dx * K + 2
                        for c in range(NCHUNK):
                            dst = ps[:, c, 0:NFLAT].rearrange(
                                "p (y x) -> p y x", x=PW)[:, :, 0:W]
                            nc.tensor.matmul(
                                dst,
                                w_tile[:, wi, :],
                                window2d(apad,
                                         (CROWS * c + 2 * DIL) * PW
                                         + DIL * dx),
                                start=False, stop=(dx == K - 1),
                            )

                # ---- fused eviction + shuffle + residual: fin (bf16) =
                # psum (keep x<32 of each 40-col row) + x_res
                fin = finp.tile([128, NHALF, S], bf16, name="fin",
                                tag="fin")
                for h in range(NHALF):
                    keep = ps_h[h][:, :, 0:NFLAT].rearrange(
                        "p c (y x) -> p c y x", x=PW)[:, :, :, 0:W]
                    nc.vector.tensor_add(
                        fin[:, h, :].rearrange("p (c y x) -> p c y x",
                                               c=NCHUNK, x=W),
                        keep,
                        xr[:, h, :].rearrange("p (c y x) -> p c y x",
                                              c=NCHUNK, x=W),
                    )

                # ---- store: scatter 32-partition blocks back to natural
                # channel order (channel 64j + 32h + g <- partition 32j+g,
                # slot h)
                for j in range(4):
                    nc.sync.dma_start(
                        out[n, 64 * j:64 * j + 64, :].rearrange(
                            "(h g) s -> g h s", h=2),
                        fin[32 * j:32 * j + 32, :, :])

    nc.compile()
    return nc


def _host_prep(x, weight, gamma, beta, running_mean, running_var):
    """Precompute BN affine + block-diagonal signed weights."""
    inv = (gamma / np.sqrt(running_var + EPS)).astype(np.float32)
    bias = (beta - running_mean * inv).astype(np.float32)
    wsign = np.sign(weight).astype(np.float32)   # [256, 4, 3, 3]

    lhsT = np.zeros((NHALF, K * K, 128, 128), np.float32)
    # Column m of lhsT (-> PSUM partition m) holds cout co = 4*(m%32)+m//32
    # within the half, so PSUM partition order is m = 32j + g for conv
    # cout 4g + j; the shuffled final channel is then 64j + 32h + g.
    m = np.arange(128)
    co = CPG * (m % 32) + m // 32
    gl = co // CPG
    for h in range(NHALF):
        for dy in range(K):
            for dx in range(K):
                # device tap index: dx*3 + dy (dy 0/1 = DoubleRow pair)
                t = dx * K + dy
                for kk in range(CPG):
                    lhsT[h, t, CPG * gl + kk, m] = wsign[128 * h + co, kk,
                                                         dy, dx]
    # device weight layout: [ci, (h,t), m], fp8, contiguous upload
    lhsT = np.ascontiguousarray(
        lhsT.astype(ml_dtypes.float8_e4m3)
        .transpose(2, 0, 1, 3)
        .reshape(128, NHALF * K * K, 128))
    sc = np.ascontiguousarray(inv.reshape(NHALF, 128))
    bi = np.ascontiguousarray(bias.reshape(NHALF, 128))
    return lhsT, sc, bi


def _get_compiled():
    global _COMPILED
    if _COMPILED is None:
        _COMPILED = build(NS)
    return _COMPILED


def make_in_maps(x, weight, gamma, beta, running_mean, running_var):
    lhsT, sc, bi = _host_prep(x, weight, gamma, beta, running_mean,
                              running_var)
    xs = x.astype(np.float32).reshape(N_CORES, NS, C, S)
    xns = np.ascontiguousarray(xs.astype(np.float16))
    xrs = np.ascontiguousarray(xs.astype(ml_dtypes.bfloat16))
    return [
        {"xin": xns[i], "xres": xrs[i], "wT": lhsT, "bnsc": sc, "bnbi": bi}
        for i in range(N_CORES)
    ]


def kernel(x, weight, gamma, beta, running_mean, running_var):
    from concourse.bass_utils import run_bass_kernel_spmd

    nc = _get_compiled()
    in_maps = make_in_maps(np.asarray(x), np.asarray(weight),
                           np.asarray(gamma), np.asarray(beta),
                           np.asarray(running_mean), np.asarray(running_var))
    res = run_bass_kernel_spmd(nc, in_maps, list(range(N_CORES)))
    outs = [res.results[i]["out"].astype(np.float32)
            .reshape(NS, C, H, W) for i in range(N_CORES)]
    return np.concatenate(outs, axis=0)
